# revision 12
# baseline (speedup 1.0000x reference)
"""Diag-embed kernel for Trainium2 (raw Bass, manual semaphores).

Problem: x [8192, 176] f32 -> out [8192, 176, 176] f32 with
out[i] = diag(x[i]).  Data-parallel over 8 NeuronCores: core c handles
batch rows [1024c, 1024(c+1)).

Per core the output block is 1024*176*176*4 B ~= 127 MB of mostly zeros
-> purely HBM-write bound.  The per-item flat row (30976 floats) is cut
into SEGMENTS column segments; a persistent SBUF template per segment
holds that segment for 128 items (partition p = chunk item p).  The zero
background is memset once; per chunk of 128 items only the diagonal
slots (flat offset j*177) are refreshed with one strided DVE copy per
segment, then each segment streams out as one large contiguous DMA.
With S segments up to S store-DMAs are in flight, hiding the per-DMA
completion latency.  Manual semaphores keep every instruction at <=1
sync wait (the TRN2 codegen rejects more).
"""

from itertools import zip_longest

import numpy as np

B_FULL = 8192
D = 176
DD = D * D            # 30976 floats per item
N_CORES = 8
B_SHARD = B_FULL // N_CORES   # 1024
P = 128
N_CHUNKS = B_SHARD // P       # 8

SEGMENTS = 8          # DD % SEGMENTS == 0; templates total 121 KiB/partition

_prog_cache = {}


def _segment_diag(s: int, W: int):
    """(j0, cnt, c0): diag indices [j0, j0+cnt) fall in columns
    [s*W, (s+1)*W) of the flat item row, at in-segment offset
    c0 + k*(D+1)."""
    j0 = -(-(s * W) // (D + 1))                 # ceil
    j1 = ((s + 1) * W - 1) // (D + 1)           # floor, inclusive
    return j0, j1 - j0 + 1, j0 * (D + 1) - s * W


def _ring_split(S: int, rings: int):
    """Assign segment indices to (sync, scalar, gpsimd) store rings."""
    if rings == 1:
        return list(range(S)), [], []
    if rings == 2:
        return list(range(S // 2)), list(range(S // 2, S)), []
    n_g = max(1, S // 4)
    n_sp = (S - n_g + 1) // 2
    return (
        list(range(n_sp)),
        list(range(n_sp, S - n_g)),
        list(range(S - n_g, S)),
    )


def _build_program(
    repeat: int = 1,
    timing: bool = False,
    segments: int = SEGMENTS,
    rings: int = 2,
    freerun: bool = False,
):
    """repeat>1 re-runs the whole store pipeline (same output region)
    inside one NEFF.  timing=True redirects the big output to an internal
    DRAM scratch tensor (same HBM-write work) and exposes only a tiny
    [128,1] ExternalOutput, so benchmarking doesn't ship 1 GB over the
    axon relay.  rings=3 adds a third store ring on the DVE HWDGE.
    freerun=True drops the per-chunk diagonal refresh entirely (output is
    all-zero -> WRONG); it exists purely as a bandwidth probe for the
    timing path.  All knobs except the defaults are for test.py only."""
    from concourse import bass, mybir

    f32 = mybir.dt.float32
    S = segments
    assert DD % S == 0
    W = DD // S
    sp_segs, act_segs, gp_segs = _ring_split(S, rings)
    nc = bass.Bass(target_bir_lowering=False)

    x = nc.dram_tensor("x", [B_SHARD, D], f32, kind="ExternalInput")
    if timing:
        out = nc.dram_tensor("outscratch", [B_SHARD, D, D], f32)
        tiny = nc.dram_tensor("tiny_out", [P, 1], f32, kind="ExternalOutput")
    else:
        out = nc.dram_tensor("out", [B_SHARD, D, D], f32, kind="ExternalOutput")
        tiny = None
    out2d = out[:].rearrange("b i j -> b (i j)")   # [1024, 30976]

    import contextlib

    with contextlib.ExitStack() as ctx:
        sem_x = ctx.enter_context(nc.semaphore("sem_x"))
        sem_t = ctx.enter_context(nc.semaphore("sem_t"))
        sem_d = [ctx.enter_context(nc.semaphore(f"sem_d{s}")) for s in range(S)]
        sem_s = [ctx.enter_context(nc.semaphore(f"sem_s{s}")) for s in range(S)]
        tmpl = [
            ctx.enter_context(nc.sbuf_tensor(f"t{s}", [P, W], f32))
            for s in range(S)
        ]
        xall = ctx.enter_context(
            nc.sbuf_tensor("xall", [P, N_CHUNKS, D], f32)
        )
        diag = [_segment_diag(s, W) for s in range(S)]

        # When GpSimd is unused, skip its expensive dge_drain in the end
        # barrier; with a gpsimd store ring it must drain normally.
        with nc.Block(no_gpsimd_drain=not gp_segs) as block:

            # stores ride multiple DGE rings (SP, ACT, optionally GpSimd):
            # when one ring's head waits on a scatter sem the others keep
            # the SDMA engines fed
            def store_stream(eng, segs):
                for m in range(N_CHUNKS * repeat):
                    n = m % N_CHUNKS
                    rows = slice(n * P, (n + 1) * P)
                    for s in segs:
                        dma = eng.dma_start(
                            out=out2d[rows, s * W : (s + 1) * W], in_=tmpl[s][:]
                        )
                        if freerun:
                            if m == 0:
                                dma.wait_op(sem_s[s], 1, "sem-ge")  # memset done
                        else:
                            dma.wait_op(sem_s[s], m + 1, "sem-ge")   # RAW: scatter
                        dma.then_inc(sem_d[s], 16)
                # all stores landed before the end-of-kernel barrier
                for s in segs:
                    eng.wait_ge(sem_d[s], 16 * N_CHUNKS * repeat)

            @block.scalar
            def _(act):
                # per-chunk x loads on the ACT HWDGE queue (parallel to the
                # store queue); chunk 0's 90 KB load unblocks the pipeline
                for n in range(N_CHUNKS):
                    act.dma_start(
                        out=xall[:, n, :], in_=x[n * P : (n + 1) * P, :]
                    ).then_inc(sem_x, 16)
                store_stream(act, act_segs)
                if freerun:
                    act.wait_ge(sem_x, 16 * N_CHUNKS)

            # m==0 emission order: round-robin across the store rings so
            # every ring's first segment is memset+scattered early (in ring
            # order each ring's 1st seg would otherwise be S/rings deep)
            order0 = []
            for tup in zip_longest(sp_segs, act_segs, gp_segs):
                order0 += [s for s in tup if s is not None]

            @block.vector
            def _(v):
                if freerun:
                    # bandwidth probe: zero-fill once, no per-chunk refresh
                    for s in range(S):
                        v.memset(tmpl[s][:], 0.0).then_inc(sem_s[s])
                    return
                for m in range(N_CHUNKS * repeat):
                    n = m % N_CHUNKS
                    for s in (order0 if m == 0 else range(S)):
                        j0, cnt, c0 = diag[s]
                        if m == 0:
                            # interleave zero-fills with the first chunk's
                            # scatters so dma_s(0) can start right after
                            # memset s instead of after all S memsets
                            v.memset(tmpl[s][:], 0.0)
                        i = v.tensor_copy(
                            tmpl[s][:, c0 : c0 + (cnt - 1) * (D + 1) + 1 : D + 1],
                            xall[:, n, j0 : j0 + cnt],
                        )
                        if m == 0:
                            if s == order0[0]:
                                i.wait_op(sem_x, 16, "sem-ge")   # chunk 0's x
                            elif s == order0[-1]:
                                # guard: every later scatter follows this one
                                # in DVE program order, so all x is resident
                                i.wait_op(sem_x, 16 * N_CHUNKS, "sem-ge")
                        else:
                            i.wait_op(sem_d[s], 16 * m, "sem-ge")  # WAR
                        i.then_inc(sem_s[s])

            if gp_segs:

                @block.gpsimd
                def _(gp):
                    store_stream(gp, gp_segs)

            @block.sync
            def _(sp):
                store_stream(sp, sp_segs)
                if tiny is not None:
                    dt_ = sp.dma_start(out=tiny[:], in_=tmpl[0][:, 0:1])
                    dt_.then_inc(sem_t, 16)
                    sp.wait_ge(sem_t, 16)

    return nc


def _get_program(
    repeat: int = 1,
    timing: bool = False,
    segments: int = SEGMENTS,
    rings: int = 2,
    freerun: bool = False,
):
    key = ("nc", repeat, timing, segments, rings, freerun)
    if key not in _prog_cache:
        _prog_cache[key] = _build_program(repeat, timing, segments, rings, freerun)
    return _prog_cache[key]


def _run(x: np.ndarray, **spmd_kwargs):
    from concourse.bass_utils import run_bass_kernel_spmd

    x = np.ascontiguousarray(x, dtype=np.float32)
    assert x.shape == (B_FULL, D), x.shape
    nc = _get_program()
    in_maps = [
        {"x": x[c * B_SHARD : (c + 1) * B_SHARD]} for c in range(N_CORES)
    ]
    res = run_bass_kernel_spmd(nc, in_maps, list(range(N_CORES)), **spmd_kwargs)
    full = np.concatenate([r["out"] for r in res.results], axis=0)
    return full, res


def kernel(**inputs) -> np.ndarray:
    full, _ = _run(inputs["x"])
    return full

